# revision 19
# baseline (speedup 1.0000x reference)
"""Distributed Trainium2 Bass kernel for GQA causal attention
(S=2048, DIM=4096, NH=32, NKV=8, HD=128), tensor-parallel over heads on 8
NeuronCores.

Per-core program (core c owns q-heads 4c..4c+3 and kv-head c):
  1. QKV projection: qT/kT/vT = W.T-slices @ x.T   (bf16 matmul, f32 psum)
  2. RoPE on q/k via a signed pair-permutation matmul + DVE combine,
     output cast to bf16
  3. PE-transpose vT -> v (bf16)
  4. Causal attention in "scores-transposed" layout, all-bf16 matmuls:
     sT[kv,q] = kT.T q, with diagonal blocks narrowed to the causally
     valid q-range; exp on ACT over PAIRS of blocks (one [128,<=1024]
     activation per two kv-blocks); causal 0/1 mask multiplied in-place
     on the one partially-valid 128-col chunk per diagonal block;
     yT[hd,q] += v.T p as a single 128-partition matmul per block;
     denominator = ones.T @ p_sum where p_sum is accumulated on the DVE.
  5. Normalize (reciprocal computed in a [128,4] layout to use all DVE
     lanes, broadcast via a ones-column matmul), cast bf16, per-head
     AllGather -> full Y.T [4096, S]
  6. Output projection, entirely after attention in si order 3,2,1,0 so
     its y_gather reads can never head-of-line-block attention matmuls:
     core c computes out[:, 512c:512(c+1)] (as outT).

Host side shards/preps inputs (transposes, bf16 casts, cos/sin/mask/perm
tables) and concatenates the 8 output column-slices.
"""

import sys

sys.path.insert(0, "/opt/trn_rl_repo")

import numpy as np
import ml_dtypes

import concourse.bass as bass
import concourse.mybir as mybir
import concourse.tile as tile
from concourse import bacc
from concourse import bass_utils

S, DIM = 2048, 4096
NH, NKV, HD = 32, 8, 128
NCORES = 8
QH = NH // NCORES  # 4 q heads per core
KT = DIM // 128  # 32 contraction tiles
ST = S // 512  # 4 sequence tiles of 512
SCALE = 1.0 / float(np.sqrt(HD))

BF = mybir.dt.bfloat16
F32 = mybir.dt.float32
F32R = mybir.dt.float32r
ALU = mybir.AluOpType
ACTF = mybir.ActivationFunctionType


def r32(ap):
    return ap.bitcast(F32R)


def build_nc():
    nc = bacc.Bacc(
        "TRN2",
        target_bir_lowering=False,
        debug=False,
        enable_asserts=True,
        num_devices=NCORES,
    )

    xt = nc.dram_tensor("xt", [DIM, S], BF, kind="ExternalInput").ap()
    wqkvt = nc.dram_tensor("wqkvt", [DIM, 768], BF, kind="ExternalInput").ap()
    wot = nc.dram_tensor("wot", [DIM, 512], BF, kind="ExternalInput").ap()
    cost = nc.dram_tensor("cost", [128, S], F32, kind="ExternalInput").ap()
    sint = nc.dram_tensor("sint", [128, S], F32, kind="ExternalInput").ap()
    maskt = nc.dram_tensor("maskt", [128, 128], BF, kind="ExternalInput").ap()
    rpermt = nc.dram_tensor("rpermt", [128, 128], F32R, kind="ExternalInput").ap()
    identt = nc.dram_tensor("identt", [128, 128], F32, kind="ExternalInput").ap()
    onest = nc.dram_tensor("onest", [128, 1], BF, kind="ExternalInput").ap()
    onescolt = nc.dram_tensor("onescolt", [1, 128], F32R, kind="ExternalInput").ap()
    outt = nc.dram_tensor("outt", [512, S], F32, kind="ExternalOutput").ap()

    with tile.TileContext(nc) as tc:
        with (
            tc.tile_pool(name="const", bufs=1) as const,
            tc.tile_pool(name="qkvsb", bufs=1) as qkvsb,
            tc.tile_pool(name="dram", bufs=1, space="DRAM") as dram,
        ):
            cos_sb = const.tile([128, S], F32)
            sin_sb = const.tile([128, S], F32)
            mask_sb = const.tile([128, 128], BF)
            rperm_sb = const.tile([128, 128], F32R)
            ident_sb = const.tile([128, 128], F32)
            ones_sb = const.tile([128, 1], BF)
            onescol_sb = const.tile([1, 128], F32R)

            def load_consts():
                nc.sync.dma_start(cos_sb, cost)
                nc.sync.dma_start(sin_sb, sint)
                nc.sync.dma_start(mask_sb, maskt)
                nc.sync.dma_start(rperm_sb, rpermt)
                nc.sync.dma_start(ident_sb, identt)
                nc.sync.dma_start(ones_sb, onest)
                nc.sync.dma_start(onescol_sb, onescolt)

            agw_in = dram.tile([128, 1], BF, tag="agwi", name="agwarm_in")
            agw_out = dram.tile(
                [NCORES * 128, 1], BF, addr_space="Shared", tag="agwo", name="agwarm_out"
            )

            # persistent activations, attention operands in bf16
            q_sb = qkvsb.tile([128, QH, S], BF)  # rope'd qT, head-major
            k_sb = qkvsb.tile([128, S], BF)  # rope'd kT
            v_sb = qkvsb.tile([128, S], BF)  # v, block-transposed

            # ---------------- phase 1: QKV projections + RoPE ----------------
            with (
                tc.tile_pool(name="wqkv", bufs=1) as wqkv,
                tc.tile_pool(name="xs", bufs=4) as xs,
                tc.tile_pool(name="stg", bufs=4) as stg,
                tc.tile_pool(name="psacc", bufs=4, space="PSUM") as psacc,
                tc.tile_pool(name="psstr", bufs=2, space="PSUM") as psstr,
            ):
                w_sb = wqkv.tile([128, KT, 768], BF)
                wqkvt_r = wqkvt.rearrange("(kb p) m -> p kb m", p=128)

                def rope_tile(stage, dst_slice, s0):
                    """stage: [128,512] f32 SBUF (pre-rope, already drained
                    from psum). dst_slice: SBUF bf16 [128,512] destination."""
                    rot = psstr.tile([128, 512], F32, tag="str")
                    nc.tensor.matmul(rot, rperm_sb, r32(stage))
                    t1 = stg.tile([128, 512], F32, tag="ropetmp")
                    nc.vector.tensor_tensor(
                        t1, stage, cos_sb[:, s0 : s0 + 512], ALU.mult
                    )
                    t2 = stg.tile([128, 512], F32, tag="ropetmp2")
                    nc.vector.tensor_tensor(
                        t2, rot, sin_sb[:, s0 : s0 + 512], ALU.mult
                    )
                    nc.vector.tensor_tensor(dst_slice, t1, t2, ALU.add)

                for si in range(ST):
                    s0 = 512 * si
                    ps = [
                        psacc.tile(
                            [128, 512],
                            F32,
                            tag="acc" if m < 4 else "oacc",
                            bufs=4 if m < 4 else 2,
                            name=f"qkv_ps_{si}_{m}",
                        )
                        for m in range(6)
                    ]
                    xt_r = xt.rearrange("(kb p) s -> p kb s", p=128)
                    for k2 in range(KT // 2):
                        if si == 0:
                            if k2 == 0:
                                # k-granular first chunk: the first matmul
                                # only needs k=0, so don't make it wait for
                                # a 2-k-tile transfer
                                for kk0 in range(2):
                                    nc.sync.dma_start(
                                        w_sb[:, kk0 : kk0 + 1, :],
                                        wqkvt_r[:, kk0 : kk0 + 1, :],
                                    )
                            else:
                                nc.sync.dma_start(
                                    w_sb[:, 2 * k2 : 2 * k2 + 2, :],
                                    wqkvt_r[:, 2 * k2 : 2 * k2 + 2, :],
                                )
                        xtile = xs.tile([128, 2, 512], BF, tag="xtile")
                        if si == 0 and k2 == 0:
                            for kk0 in range(2):
                                nc.sync.dma_start(
                                    xtile[:, kk0 : kk0 + 1, :],
                                    xt_r[:, kk0 : kk0 + 1, s0 : s0 + 512],
                                )
                        else:
                            nc.sync.dma_start(
                                xtile, xt_r[:, 2 * k2 : 2 * k2 + 2, s0 : s0 + 512]
                            )
                        for kk in range(2):
                            k = 2 * k2 + kk
                            for m in range(6):
                                nc.tensor.matmul(
                                    ps[m],
                                    w_sb[:, k, 128 * m : 128 * (m + 1)],
                                    xtile[:, kk, :],
                                    start=(k == 0),
                                    stop=(k == KT - 1),
                                )
                    if si == 0:
                        load_consts()
                        # tiny warm-up AllGather: absorbs first-collective
                        # overhead during QKV so the first real AG is fast
                        nc.sync.dma_start(agw_in, onest)
                        nc.gpsimd.collective_compute(
                            "AllGather",
                            ALU.bypass,
                            ins=[agw_in.opt()],
                            outs=[agw_out.opt()],
                            replica_groups=[list(range(NCORES))],
                        )
                    # drain ALL 6 psum accumulators to SBUF first so the next
                    # si's matmuls get their psum banks back immediately; the
                    # rope math then runs off the SBUF stage tiles. For the
                    # LAST si there is no next si to unblock, so interleave
                    # drain+rope per head instead: attention's first score
                    # pair only needs q head 0, which becomes ready ~3us
                    # sooner.
                    stages = []
                    for m in range(6):
                        stage = stg.tile(
                            [128, 512], F32, tag="stage", bufs=8, name=f"stage_{si}_{m}"
                        )
                        stages.append(stage)
                    if si < ST - 1:
                        for m in range(6):
                            nc.vector.tensor_copy(r32(stages[m]), ps[m])
                        for m in range(QH):
                            rope_tile(stages[m], q_sb[:, m, s0 : s0 + 512], s0)
                        rope_tile(stages[QH], k_sb[:, s0 : s0 + 512], s0)
                    else:
                        for m in range(QH):
                            nc.vector.tensor_copy(r32(stages[m]), ps[m])
                            rope_tile(stages[m], q_sb[:, m, s0 : s0 + 512], s0)
                        nc.vector.tensor_copy(r32(stages[QH]), ps[QH])
                        rope_tile(stages[QH], k_sb[:, s0 : s0 + 512], s0)
                        nc.vector.tensor_copy(r32(stages[QH + 1]), ps[QH + 1])
                    # v: 4 PE transposes from the stage -> v_sb bf16
                    vstage = stages[QH + 1]
                    for jj in range(4):
                        j = 4 * si + jj
                        vt_ps = psstr.tile([128, 128], F32, tag="str")
                        nc.tensor.transpose(
                            vt_ps, vstage[:, 128 * jj : 128 * (jj + 1)], ident_sb
                        )
                        nc.vector.tensor_copy(
                            v_sb[:, 128 * j : 128 * (j + 1)], vt_ps
                        )

            # ---------------- phases 3-5: attention, normalize, allgather ----
            y_bounce = {
                qt: dram.tile(
                    [QH * 128, 512], BF, tag=f"yb{qt}", name=f"ybounce{qt}"
                )
                for qt in range(ST)
            }
            y_gather = {
                qt: dram.tile(
                    [NCORES * QH * 128, 512],
                    BF,
                    addr_space="Shared",
                    tag=f"yg{qt}",
                    name=f"ygather{qt}",
                )
                for qt in range(ST)
            }

            with (
                tc.tile_pool(name="pp", bufs=4) as pp,
                tc.tile_pool(name="psm", bufs=2) as psm,
                tc.tile_pool(name="nrm", bufs=3) as nrm,
                tc.tile_pool(name="wo", bufs=1) as wo,
                tc.tile_pool(name="ys", bufs=4) as ys,
                tc.tile_pool(name="osb", bufs=4) as osb,
                tc.tile_pool(name="psc", bufs=2, space="PSUM") as psc,
                tc.tile_pool(name="psy", bufs=2, space="PSUM") as psy,
            ):
                # wo weights DMA'd here so the transfer overlaps attention
                wo_sb = wo.tile([128, KT, 512], BF)
                wot_r = wot.rearrange("(kb p) m -> p kb m", p=128)
                for k4 in range(KT // 4):
                    nc.sync.dma_start(
                        wo_sb[:, 4 * k4 : 4 * k4 + 4, :],
                        wot_r[:, 4 * k4 : 4 * k4 + 4, :],
                    )

                def normalize(st):
                    # deferred epilogue: runs one (qt,h) group later so the
                    # reciprocal round-trip never head-of-line-blocks the PE
                    yraw, den_sb, nh, nqt = st
                    den_t = nrm.tile([128, 4], F32, tag="dent")
                    nc.sync.dma_start(den_t, den_sb)
                    rec_t = nrm.tile([128, 4], F32R, tag="rect")
                    with nc.allow_low_precision(reason="f32r for bcast matmul"):
                        nc.vector.reciprocal(rec_t, den_t)
                    rec_sb = nrm.tile([1, 512], F32R, tag="recsb")
                    nc.sync.dma_start(rec_sb, rec_t)
                    bc_ps = psy.tile(
                        [128, 512], F32, tag="den", bufs=2, name=f"bc_{nh}_{nqt}"
                    )
                    nc.tensor.matmul(bc_ps, onescol_sb, rec_sb)
                    yn = nrm.tile([128, 512], BF, tag="yn")
                    nc.vector.tensor_tensor(yn, yraw, bc_ps, ALU.mult)
                    nc.sync.dma_start(
                        y_bounce[nqt][128 * nh : 128 * (nh + 1), :], yn
                    )
                    if nh == QH - 1:
                        nc.gpsimd.collective_compute(
                            "AllGather",
                            ALU.bypass,
                            ins=[y_bounce[nqt].opt()],
                            outs=[y_gather[nqt].opt()],
                            replica_groups=[list(range(NCORES))],
                        )

                pending = None
                epi = None

                def make_epi(y_ps, p_sum, h, qt):
                    # group epilogue, deferred until after the first score
                    # pair of the NEXT group is in the PE stream: the den
                    # matmul waits on the DVE p_sum chain, and emitting it
                    # last keeps it from head-of-line-blocking the next
                    # group's score matmuls (which would starve the ACT exp
                    # pipeline at every group boundary).
                    def run():
                        nonlocal pending
                        den_ps = psy.tile(
                            [1, 512], F32, tag="den", bufs=2, name=f"den_{h}_{qt}"
                        )
                        nc.tensor.matmul(den_ps, ones_sb, p_sum)
                        yraw = nrm.tile([128, 512], F32, tag="yraw")
                        nc.vector.tensor_copy(yraw, y_ps)
                        den_sb = nrm.tile([1, 512], F32, tag="densb")
                        nc.vector.tensor_copy(den_sb, den_ps)
                        if pending is not None:
                            normalize(pending)
                        pending = (yraw, den_sb, h, qt)

                    return run

                # flat software pipeline over all (qt, h, pair) steps: each
                # step's score matmuls are emitted one step AHEAD of its
                # exp/mask/y stage, so the ACT exp pipeline never bubbles at
                # group boundaries.
                flat = []
                for qt in reversed(range(ST)):
                    nb = 4 * qt + 4
                    for h in range(QH):
                        for pi in range(nb // 2):
                            flat.append((qt, h, pi, nb))

                sc_pend = {}
                state = {}

                def emit_scores(qt, h, pi, nb):
                    s0 = 512 * qt
                    js = (2 * pi, 2 * pi + 1)
                    offs = tuple(max(0, 128 * (j - 4 * qt)) for j in js)
                    sc = psc.tile(
                        [128, 1024], F32, tag="sc", name=f"sc_{h}_{qt}_{pi}"
                    )
                    for idx, j in enumerate(js):
                        o = offs[idx]
                        nc.tensor.matmul(
                            sc[:, 512 * idx + o : 512 * idx + 512],
                            k_sb[:, 128 * j : 128 * (j + 1)],
                            q_sb[:, h, s0 + o : s0 + 512],
                        )
                    sc_pend[(qt, h, pi)] = (sc, js, offs)

                def emit_rest(qt, h, pi, nb):
                    s0 = 512 * qt
                    sc, js, offs = sc_pend.pop((qt, h, pi))
                    if pi == 0:
                        state[(qt, h)] = (
                            psy.tile(
                                [128, 512], F32, tag="yacc", bufs=2, name=f"y_{h}_{qt}"
                            ),
                            psm.tile(
                                [128, 512], BF, tag="ps", name=f"psum_{h}_{qt}"
                            ),
                        )
                    y_ps, p_sum = state[(qt, h)]
                    u = offs[0]
                    p = pp.tile([128, 1024], BF, tag="p")
                    nc.scalar.activation(
                        p[:, u:1024], sc[:, u:1024], ACTF.Exp, scale=SCALE
                    )
                    for idx, j in enumerate(js):
                        o = offs[idx]
                        base = 512 * idx
                        if j >= 4 * qt:
                            # in-place causal mask on the one partially
                            # valid 128-col chunk of this diagonal block
                            nc.vector.tensor_tensor(
                                p[:, base + o : base + o + 128],
                                p[:, base + o : base + o + 128],
                                mask_sb,
                                ALU.mult,
                            )
                        seg = p[:, base + o : base + 512]
                        if j == 0:
                            nc.vector.tensor_copy(p_sum, seg)
                        else:
                            nc.vector.tensor_tensor(
                                p_sum[:, o:512], p_sum[:, o:512], seg, ALU.add
                            )
                        nc.tensor.matmul(
                            y_ps[:, o:512],
                            v_sb[:, 128 * j : 128 * (j + 1)],
                            seg,
                            start=(j == 0),
                            stop=(j == nb - 1),
                        )

                for i, step in enumerate(flat):
                    if i == 0:
                        emit_scores(*step)
                    if i + 1 < len(flat):
                        emit_scores(*flat[i + 1])
                    emit_rest(*step)
                    qt, h, pi, nb = step
                    if pi == 0 and epi is not None:
                        epi()
                        epi = None
                    if pi == nb // 2 - 1:
                        y_ps, p_sum = state.pop((qt, h))
                        epi = make_epi(y_ps, p_sum, h, qt)
                epi()
                normalize(pending)

                # ---------------- phase 6: output projection ----------------
                # tile_wait_until pins these instructions after ALL attention
                # work in the scheduler's per-engine streams: a y_gather read
                # (which blocks on the AllGather semaphore) must never be
                # hoisted ahead of attention DMAs/matmuls (head-of-line
                # blocking on the in-order engine queues).
                for osi, si in enumerate([3, 2, 1, 0]):
                    tc.tile_set_cur_wait(1.0 + 0.1 * osi)
                    s0 = 512 * si
                    ops = {}
                    for oc in range(4):
                        ops[oc] = psy.tile(
                            [128, 512],
                            F32,
                            tag="yacc" if oc < 2 else "den",
                            bufs=2,
                            name=f"o_ps_{si}_{oc}",
                        )
                    # ytile loads go on the Scalar engine's DMA queue (idle
                    # after attention) with a deep prefetch ring, so neither
                    # the sync queue's normalize DMAs nor AllGather SDMA
                    # traffic contending for HBM can starve the PE; batched 2
                    # k-tiles per DMA to halve issue cost.
                    yg_r = y_gather[si].rearrange("(kb p) s -> p kb s", p=128)
                    for k4 in range(KT // 4):
                        ytile = ys.tile([128, 4, 512], BF, tag="ytile", bufs=4)
                        nc.scalar.dma_start(
                            ytile, yg_r[:, 4 * k4 : 4 * k4 + 4, :]
                        )
                        for kk in range(4):
                            ki = 4 * k4 + kk
                            for oc in range(4):
                                nc.tensor.matmul(
                                    ops[oc],
                                    wo_sb[:, ki, 128 * oc : 128 * (oc + 1)],
                                    ytile[:, kk, :],
                                    start=(ki == 0),
                                    stop=(ki == KT - 1),
                                )
                    for oc in range(4):
                        otile = osb.tile([128, 512], F32, tag="otile")
                        nc.vector.tensor_copy(otile, ops[oc])
                        nc.sync.dma_start(
                            outt[128 * oc : 128 * (oc + 1), s0 : s0 + 512],
                            otile,
                        )

    nc.compile()
    return nc


def make_in_maps(x, freqs_cis, wq, wk, wv, wo):
    f32 = np.float32
    bf = ml_dtypes.bfloat16
    xt = np.ascontiguousarray(x.T).astype(bf)
    cos = np.ascontiguousarray(np.repeat(freqs_cis[:, :, 0].T, 2, axis=0)).astype(f32)
    sin = np.ascontiguousarray(np.repeat(freqs_cis[:, :, 1].T, 2, axis=0)).astype(f32)
    kvi = np.arange(128, dtype=np.int64)[:, None]
    qi = np.arange(128, dtype=np.int64)[None, :]
    mask = (kvi <= qi).astype(f32).astype(bf)  # [128, 128] lower-tri incl diag
    rperm = np.zeros((128, 128), f32)
    for r in range(64):
        rperm[2 * r, 2 * r + 1] = -1.0
        rperm[2 * r + 1, 2 * r] = 1.0
    rpermT = np.ascontiguousarray(rperm.T)
    ident = np.eye(128, dtype=f32)
    ones = np.ones((128, 1), bf)
    onescol = np.ones((1, 128), f32)

    in_maps = []
    for c in range(NCORES):
        wqkv = np.concatenate(
            [
                wq[512 * c : 512 * (c + 1), :].T,
                wk[128 * c : 128 * (c + 1), :].T,
                wv[128 * c : 128 * (c + 1), :].T,
            ],
            axis=1,
        ).astype(bf)  # [DIM, 768]
        wot = np.ascontiguousarray(wo[512 * c : 512 * (c + 1), :].T).astype(bf)
        in_maps.append(
            {
                "xt": xt,
                "wqkvt": np.ascontiguousarray(wqkv),
                "wot": wot,
                "cost": cos,
                "sint": sin,
                "maskt": np.ascontiguousarray(mask),
                "rpermt": rpermT,
                "identt": ident,
                "onest": ones,
                "onescolt": onescol,
            }
        )
    return in_maps


def install_ntff_hook():
    """Inject the missing ``antenv.axon_hooks`` module backed by ctypes calls
    into libaxon_pjrt.so, enabling run_bass_kernel_spmd(trace=True) under
    axon. Also neuter upload_artifacts (no artifact bucket here)."""
    import sys as _sys
    import types
    import ctypes
    import contextlib

    if "antenv.axon_hooks" in _sys.modules:
        return
    so_path = "/opt/axon/libaxon_pjrt.so"
    lib = ctypes.CDLL(so_path)
    lib.axon_start_nrt_profile.argtypes = [
        ctypes.POINTER(ctypes.c_int64),
        ctypes.c_size_t,
    ]
    lib.axon_start_nrt_profile.restype = ctypes.c_int64
    lib.axon_stop_nrt_profile.argtypes = [ctypes.c_char_p]
    lib.axon_stop_nrt_profile.restype = ctypes.c_int64

    @contextlib.contextmanager
    def _hook(output_dir, device_ids):
        import jax

        jax.devices()
        if device_ids:
            ids = (ctypes.c_int64 * len(device_ids))(*device_ids)
            rc = lib.axon_start_nrt_profile(ids, len(device_ids))
        else:
            rc = lib.axon_start_nrt_profile(None, 0)
        if rc != 0:
            raise RuntimeError(f"axon_start_nrt_profile rc={rc}")
        try:
            yield
        finally:
            n = lib.axon_stop_nrt_profile(str(output_dir).encode())
            print(f"ntff profile: {n} file(s) written to {output_dir}")

    mod = types.ModuleType("antenv.axon_hooks")
    mod.get_axon_ntff_profile_hook = lambda: _hook
    mod.set_axon_ntff_profile_hook = lambda h: None
    _sys.modules["antenv.axon_hooks"] = mod
    import antenv

    antenv.axon_hooks = mod
    bass_utils.upload_artifacts = lambda tmpdir: tmpdir


def run(x, freqs_cis, wq, wk, wv, wo, trace=False, trace_kwargs=None):
    if trace:
        install_ntff_hook()
    nc = build_nc()
    in_maps = make_in_maps(x, freqs_cis, wq, wk, wv, wo)
    res = bass_utils.run_bass_kernel_spmd(
        nc,
        in_maps,
        core_ids=list(range(NCORES)),
        trace=trace,
        **(trace_kwargs or {}),
    )
    outs = [r["outt"] for r in res.results]  # each [512, S] = outT slice
    full = np.concatenate([np.asarray(o).T for o in outs], axis=1).astype(np.float32)
    return full, res


def kernel(x, freqs_cis, wq, wk, wv, wo):
    full, _ = run(
        np.asarray(x, np.float32),
        np.asarray(freqs_cis, np.float32),
        np.asarray(wq, np.float32),
        np.asarray(wk, np.float32),
        np.asarray(wv, np.float32),
        np.asarray(wo, np.float32),
    )
    return full


# revision 20
# speedup vs baseline: 1.0213x; 1.0213x over previous
"""Distributed Trainium2 Bass kernel for GQA causal attention
(S=2048, DIM=4096, NH=32, NKV=8, HD=128), tensor-parallel over heads on 8
NeuronCores.

Per-core program (core c owns q-heads 4c..4c+3 and kv-head c):
  1. QKV projection: qT/kT/vT = W.T-slices @ x.T   (bf16 matmul, f32 psum)
  2. RoPE on q/k via a signed pair-permutation matmul + DVE combine,
     output cast to bf16
  3. PE-transpose vT -> v (bf16)
  4. Causal attention in "scores-transposed" layout, all-bf16 matmuls:
     sT[kv,q] = kT.T q, with diagonal blocks narrowed to the causally
     valid q-range; exp on ACT over PAIRS of blocks (one [128,<=1024]
     activation per two kv-blocks); causal 0/1 mask multiplied in-place
     on the one partially-valid 128-col chunk per diagonal block;
     yT[hd,q] += v.T p as a single 128-partition matmul per block;
     denominator = ones.T @ p_sum where p_sum is accumulated on the DVE.
  5. Normalize (reciprocal computed in a [128,4] layout to use all DVE
     lanes, broadcast via a ones-column matmul), cast bf16, per-head
     AllGather -> full Y.T [4096, S]
  6. Output projection, entirely after attention in si order 3,2,1,0 so
     its y_gather reads can never head-of-line-block attention matmuls:
     core c computes out[:, 512c:512(c+1)] (as outT).

Host side shards/preps inputs (transposes, bf16 casts, cos/sin/mask/perm
tables) and concatenates the 8 output column-slices.
"""

import sys

sys.path.insert(0, "/opt/trn_rl_repo")

import numpy as np
import ml_dtypes

import concourse.bass as bass
import concourse.mybir as mybir
import concourse.tile as tile
from concourse import bacc
from concourse import bass_utils

S, DIM = 2048, 4096
NH, NKV, HD = 32, 8, 128
NCORES = 8
QH = NH // NCORES  # 4 q heads per core
KT = DIM // 128  # 32 contraction tiles
ST = S // 512  # 4 sequence tiles of 512
SCALE = 1.0 / float(np.sqrt(HD))

BF = mybir.dt.bfloat16
F32 = mybir.dt.float32
F32R = mybir.dt.float32r
ALU = mybir.AluOpType
ACTF = mybir.ActivationFunctionType


def r32(ap):
    return ap.bitcast(F32R)


def build_nc():
    nc = bacc.Bacc(
        "TRN2",
        target_bir_lowering=False,
        debug=False,
        enable_asserts=True,
        num_devices=NCORES,
    )

    xt = nc.dram_tensor("xt", [DIM, S], BF, kind="ExternalInput").ap()
    wqkvt = nc.dram_tensor("wqkvt", [DIM, 768], BF, kind="ExternalInput").ap()
    wot = nc.dram_tensor("wot", [DIM, 512], BF, kind="ExternalInput").ap()
    cost = nc.dram_tensor("cost", [128, S], F32, kind="ExternalInput").ap()
    sint = nc.dram_tensor("sint", [128, S], F32, kind="ExternalInput").ap()
    maskt = nc.dram_tensor("maskt", [128, 128], BF, kind="ExternalInput").ap()
    rpermt = nc.dram_tensor("rpermt", [128, 128], F32R, kind="ExternalInput").ap()
    identt = nc.dram_tensor("identt", [128, 128], F32, kind="ExternalInput").ap()
    onest = nc.dram_tensor("onest", [128, 1], BF, kind="ExternalInput").ap()
    onescolt = nc.dram_tensor("onescolt", [1, 128], F32R, kind="ExternalInput").ap()
    outt = nc.dram_tensor("outt", [512, S], F32, kind="ExternalOutput").ap()

    with tile.TileContext(nc) as tc:
        with (
            tc.tile_pool(name="const", bufs=1) as const,
            tc.tile_pool(name="qkvsb", bufs=1) as qkvsb,
            tc.tile_pool(name="dram", bufs=1, space="DRAM") as dram,
        ):
            cos_sb = const.tile([128, S], F32)
            sin_sb = const.tile([128, S], F32)
            mask_sb = const.tile([128, 128], BF)
            rperm_sb = const.tile([128, 128], F32R)
            ident_sb = const.tile([128, 128], F32)
            ones_sb = const.tile([128, 1], BF)
            onescol_sb = const.tile([1, 128], F32R)

            def load_consts():
                nc.sync.dma_start(cos_sb, cost)
                nc.sync.dma_start(sin_sb, sint)
                nc.sync.dma_start(mask_sb, maskt)
                nc.sync.dma_start(rperm_sb, rpermt)
                nc.sync.dma_start(ident_sb, identt)
                nc.sync.dma_start(ones_sb, onest)
                nc.sync.dma_start(onescol_sb, onescolt)

            agw_in = dram.tile([128, 1], BF, tag="agwi", name="agwarm_in")
            agw_out = dram.tile(
                [NCORES * 128, 1], BF, addr_space="Shared", tag="agwo", name="agwarm_out"
            )

            # persistent activations, attention operands in bf16
            q_sb = qkvsb.tile([128, QH, S], BF)  # rope'd qT, head-major
            k_sb = qkvsb.tile([128, S], BF)  # rope'd kT
            v_sb = qkvsb.tile([128, S], BF)  # v, block-transposed

            # ---------------- phase 1: QKV projections + RoPE ----------------
            with (
                tc.tile_pool(name="wqkv", bufs=1) as wqkv,
                tc.tile_pool(name="xs", bufs=4) as xs,
                tc.tile_pool(name="stg", bufs=4) as stg,
                tc.tile_pool(name="psacc", bufs=4, space="PSUM") as psacc,
                tc.tile_pool(name="psstr", bufs=2, space="PSUM") as psstr,
            ):
                w_sb = wqkv.tile([128, KT, 768], BF)
                wqkvt_r = wqkvt.rearrange("(kb p) m -> p kb m", p=128)

                def rope_tile(stage, dst_slice, s0):
                    """stage: [128,512] f32 SBUF (pre-rope, already drained
                    from psum). dst_slice: SBUF bf16 [128,512] destination."""
                    rot = psstr.tile([128, 512], F32, tag="str")
                    nc.tensor.matmul(rot, rperm_sb, r32(stage))
                    t1 = stg.tile([128, 512], F32, tag="ropetmp")
                    nc.vector.tensor_tensor(
                        t1, stage, cos_sb[:, s0 : s0 + 512], ALU.mult
                    )
                    t2 = stg.tile([128, 512], F32, tag="ropetmp2")
                    nc.vector.tensor_tensor(
                        t2, rot, sin_sb[:, s0 : s0 + 512], ALU.mult
                    )
                    nc.vector.tensor_tensor(dst_slice, t1, t2, ALU.add)

                for si in range(ST):
                    s0 = 512 * si
                    ps = [
                        psacc.tile(
                            [128, 512],
                            F32,
                            tag="acc" if m < 4 else "oacc",
                            bufs=4 if m < 4 else 2,
                            name=f"qkv_ps_{si}_{m}",
                        )
                        for m in range(6)
                    ]
                    xt_r = xt.rearrange("(kb p) s -> p kb s", p=128)
                    for k2 in range(KT // 2):
                        if si == 0:
                            nc.sync.dma_start(
                                w_sb[:, 2 * k2 : 2 * k2 + 2, :],
                                wqkvt_r[:, 2 * k2 : 2 * k2 + 2, :],
                            )
                        xtile = xs.tile([128, 2, 512], BF, tag="xtile")
                        nc.sync.dma_start(
                            xtile, xt_r[:, 2 * k2 : 2 * k2 + 2, s0 : s0 + 512]
                        )
                        for kk in range(2):
                            k = 2 * k2 + kk
                            for m in range(6):
                                nc.tensor.matmul(
                                    ps[m],
                                    w_sb[:, k, 128 * m : 128 * (m + 1)],
                                    xtile[:, kk, :],
                                    start=(k == 0),
                                    stop=(k == KT - 1),
                                )
                    if si == 0:
                        load_consts()
                        # tiny warm-up AllGather: absorbs first-collective
                        # overhead during QKV so the first real AG is fast
                        nc.sync.dma_start(agw_in, onest)
                        nc.gpsimd.collective_compute(
                            "AllGather",
                            ALU.bypass,
                            ins=[agw_in.opt()],
                            outs=[agw_out.opt()],
                            replica_groups=[list(range(NCORES))],
                        )
                    # drain ALL 6 psum accumulators to SBUF first so the next
                    # si's matmuls get their psum banks back immediately; the
                    # rope math then runs off the SBUF stage tiles. For the
                    # LAST si there is no next si to unblock, so interleave
                    # drain+rope per head instead: attention's first score
                    # pair only needs q head 0, which becomes ready ~3us
                    # sooner.
                    stages = []
                    for m in range(6):
                        stage = stg.tile(
                            [128, 512], F32, tag="stage", bufs=8, name=f"stage_{si}_{m}"
                        )
                        stages.append(stage)
                    if si < ST - 1:
                        for m in range(6):
                            nc.vector.tensor_copy(r32(stages[m]), ps[m])
                        for m in range(QH):
                            rope_tile(stages[m], q_sb[:, m, s0 : s0 + 512], s0)
                        rope_tile(stages[QH], k_sb[:, s0 : s0 + 512], s0)
                    else:
                        for m in range(QH):
                            nc.vector.tensor_copy(r32(stages[m]), ps[m])
                            rope_tile(stages[m], q_sb[:, m, s0 : s0 + 512], s0)
                        nc.vector.tensor_copy(r32(stages[QH]), ps[QH])
                        rope_tile(stages[QH], k_sb[:, s0 : s0 + 512], s0)
                        nc.vector.tensor_copy(r32(stages[QH + 1]), ps[QH + 1])
                    # v: 4 PE transposes from the stage -> v_sb bf16
                    vstage = stages[QH + 1]
                    for jj in range(4):
                        j = 4 * si + jj
                        vt_ps = psstr.tile([128, 128], F32, tag="str")
                        nc.tensor.transpose(
                            vt_ps, vstage[:, 128 * jj : 128 * (jj + 1)], ident_sb
                        )
                        nc.vector.tensor_copy(
                            v_sb[:, 128 * j : 128 * (j + 1)], vt_ps
                        )

            # ---------------- phases 3-5: attention, normalize, allgather ----
            y_bounce = {
                qt: dram.tile(
                    [QH * 128, 512], BF, tag=f"yb{qt}", name=f"ybounce{qt}"
                )
                for qt in range(ST)
            }
            y_gather = {
                qt: dram.tile(
                    [NCORES * QH * 128, 512],
                    BF,
                    addr_space="Shared",
                    tag=f"yg{qt}",
                    name=f"ygather{qt}",
                )
                for qt in range(ST)
            }

            with (
                tc.tile_pool(name="pp", bufs=4) as pp,
                tc.tile_pool(name="psm", bufs=2) as psm,
                tc.tile_pool(name="nrm", bufs=3) as nrm,
                tc.tile_pool(name="wo", bufs=1) as wo,
                tc.tile_pool(name="ys", bufs=4) as ys,
                tc.tile_pool(name="osb", bufs=4) as osb,
                tc.tile_pool(name="psc", bufs=2, space="PSUM") as psc,
                tc.tile_pool(name="psy", bufs=2, space="PSUM") as psy,
            ):
                # wo weights DMA'd here so the transfer overlaps attention
                wo_sb = wo.tile([128, KT, 512], BF)
                wot_r = wot.rearrange("(kb p) m -> p kb m", p=128)
                for k4 in range(KT // 4):
                    nc.sync.dma_start(
                        wo_sb[:, 4 * k4 : 4 * k4 + 4, :],
                        wot_r[:, 4 * k4 : 4 * k4 + 4, :],
                    )

                def normalize(st):
                    # deferred epilogue: runs one (qt,h) group later so the
                    # reciprocal round-trip never head-of-line-blocks the PE
                    yraw, den_sb, nh, nqt = st
                    den_t = nrm.tile([128, 4], F32, tag="dent")
                    nc.sync.dma_start(den_t, den_sb)
                    rec_t = nrm.tile([128, 4], F32R, tag="rect")
                    with nc.allow_low_precision(reason="f32r for bcast matmul"):
                        nc.vector.reciprocal(rec_t, den_t)
                    rec_sb = nrm.tile([1, 512], F32R, tag="recsb")
                    nc.sync.dma_start(rec_sb, rec_t)
                    bc_ps = psy.tile(
                        [128, 512], F32, tag="den", bufs=2, name=f"bc_{nh}_{nqt}"
                    )
                    nc.tensor.matmul(bc_ps, onescol_sb, rec_sb)
                    yn = nrm.tile([128, 512], BF, tag="yn")
                    nc.vector.tensor_tensor(yn, yraw, bc_ps, ALU.mult)
                    nc.sync.dma_start(
                        y_bounce[nqt][128 * nh : 128 * (nh + 1), :], yn
                    )
                    if nh == QH - 1:
                        nc.gpsimd.collective_compute(
                            "AllGather",
                            ALU.bypass,
                            ins=[y_bounce[nqt].opt()],
                            outs=[y_gather[nqt].opt()],
                            replica_groups=[list(range(NCORES))],
                        )

                pending = None
                epi = None

                def make_epi(y_ps, p_sum, h, qt):
                    # group epilogue, deferred until after the first score
                    # pair of the NEXT group is in the PE stream: the den
                    # matmul waits on the DVE p_sum chain, and emitting it
                    # last keeps it from head-of-line-blocking the next
                    # group's score matmuls (which would starve the ACT exp
                    # pipeline at every group boundary).
                    def run():
                        nonlocal pending
                        den_ps = psy.tile(
                            [1, 512], F32, tag="den", bufs=2, name=f"den_{h}_{qt}"
                        )
                        nc.tensor.matmul(den_ps, ones_sb, p_sum)
                        yraw = nrm.tile([128, 512], F32, tag="yraw")
                        nc.vector.tensor_copy(yraw, y_ps)
                        den_sb = nrm.tile([1, 512], F32, tag="densb")
                        nc.vector.tensor_copy(den_sb, den_ps)
                        if pending is not None:
                            normalize(pending)
                        pending = (yraw, den_sb, h, qt)

                    return run

                # flat software pipeline over all (qt, h, pair) steps: each
                # step's score matmuls are emitted one step AHEAD of its
                # exp/mask/y stage, so the ACT exp pipeline never bubbles at
                # group boundaries.
                flat = []
                for qt in reversed(range(ST)):
                    nb = 4 * qt + 4
                    for h in range(QH):
                        for pi in range(nb // 2):
                            flat.append((qt, h, pi, nb))

                sc_pend = {}
                state = {}

                def emit_scores(qt, h, pi, nb):
                    s0 = 512 * qt
                    js = (2 * pi, 2 * pi + 1)
                    offs = tuple(max(0, 128 * (j - 4 * qt)) for j in js)
                    sc = psc.tile(
                        [128, 1024], F32, tag="sc", name=f"sc_{h}_{qt}_{pi}"
                    )
                    for idx, j in enumerate(js):
                        o = offs[idx]
                        nc.tensor.matmul(
                            sc[:, 512 * idx + o : 512 * idx + 512],
                            k_sb[:, 128 * j : 128 * (j + 1)],
                            q_sb[:, h, s0 + o : s0 + 512],
                        )
                    sc_pend[(qt, h, pi)] = (sc, js, offs)

                def emit_rest(qt, h, pi, nb):
                    s0 = 512 * qt
                    sc, js, offs = sc_pend.pop((qt, h, pi))
                    if pi == 0:
                        state[(qt, h)] = (
                            psy.tile(
                                [128, 512], F32, tag="yacc", bufs=2, name=f"y_{h}_{qt}"
                            ),
                            psm.tile(
                                [128, 512], BF, tag="ps", name=f"psum_{h}_{qt}"
                            ),
                        )
                    y_ps, p_sum = state[(qt, h)]
                    u = offs[0]
                    p = pp.tile([128, 1024], BF, tag="p")
                    nc.scalar.activation(
                        p[:, u:1024], sc[:, u:1024], ACTF.Exp, scale=SCALE
                    )
                    for idx, j in enumerate(js):
                        o = offs[idx]
                        base = 512 * idx
                        if j >= 4 * qt:
                            # in-place causal mask on the one partially
                            # valid 128-col chunk of this diagonal block
                            nc.vector.tensor_tensor(
                                p[:, base + o : base + o + 128],
                                p[:, base + o : base + o + 128],
                                mask_sb,
                                ALU.mult,
                            )
                        seg = p[:, base + o : base + 512]
                        if j == 0:
                            nc.vector.tensor_copy(p_sum, seg)
                        else:
                            nc.vector.tensor_tensor(
                                p_sum[:, o:512], p_sum[:, o:512], seg, ALU.add
                            )
                        nc.tensor.matmul(
                            y_ps[:, o:512],
                            v_sb[:, 128 * j : 128 * (j + 1)],
                            seg,
                            start=(j == 0),
                            stop=(j == nb - 1),
                        )

                for i, step in enumerate(flat):
                    if i == 0:
                        emit_scores(*step)
                    if i + 1 < len(flat):
                        emit_scores(*flat[i + 1])
                    emit_rest(*step)
                    qt, h, pi, nb = step
                    if pi == 0 and epi is not None:
                        epi()
                        epi = None
                    if pi == nb // 2 - 1:
                        y_ps, p_sum = state.pop((qt, h))
                        epi = make_epi(y_ps, p_sum, h, qt)
                epi()
                normalize(pending)

                # ---------------- phase 6: output projection ----------------
                # tile_wait_until pins these instructions after ALL attention
                # work in the scheduler's per-engine streams: a y_gather read
                # (which blocks on the AllGather semaphore) must never be
                # hoisted ahead of attention DMAs/matmuls (head-of-line
                # blocking on the in-order engine queues).
                for osi, si in enumerate([3, 2, 1, 0]):
                    tc.tile_set_cur_wait(1.0 + 0.1 * osi)
                    s0 = 512 * si
                    ops = {}
                    for oc in range(4):
                        ops[oc] = psy.tile(
                            [128, 512],
                            F32,
                            tag="yacc" if oc < 2 else "den",
                            bufs=2,
                            name=f"o_ps_{si}_{oc}",
                        )
                    # ytile loads go on the Scalar engine's DMA queue (idle
                    # after attention) with a deep prefetch ring, so neither
                    # the sync queue's normalize DMAs nor AllGather SDMA
                    # traffic contending for HBM can starve the PE; batched 2
                    # k-tiles per DMA to halve issue cost.
                    yg_r = y_gather[si].rearrange("(kb p) s -> p kb s", p=128)
                    for k4 in range(KT // 4):
                        ytile = ys.tile([128, 4, 512], BF, tag="ytile", bufs=4)
                        nc.scalar.dma_start(
                            ytile, yg_r[:, 4 * k4 : 4 * k4 + 4, :]
                        )
                        for kk in range(4):
                            ki = 4 * k4 + kk
                            for oc in range(4):
                                nc.tensor.matmul(
                                    ops[oc],
                                    wo_sb[:, ki, 128 * oc : 128 * (oc + 1)],
                                    ytile[:, kk, :],
                                    start=(ki == 0),
                                    stop=(ki == KT - 1),
                                )
                    for oc in range(4):
                        otile = osb.tile([128, 512], F32, tag="otile")
                        nc.vector.tensor_copy(otile, ops[oc])
                        nc.sync.dma_start(
                            outt[128 * oc : 128 * (oc + 1), s0 : s0 + 512],
                            otile,
                        )

    nc.compile()
    return nc


def make_in_maps(x, freqs_cis, wq, wk, wv, wo):
    f32 = np.float32
    bf = ml_dtypes.bfloat16
    xt = np.ascontiguousarray(x.T).astype(bf)
    cos = np.ascontiguousarray(np.repeat(freqs_cis[:, :, 0].T, 2, axis=0)).astype(f32)
    sin = np.ascontiguousarray(np.repeat(freqs_cis[:, :, 1].T, 2, axis=0)).astype(f32)
    kvi = np.arange(128, dtype=np.int64)[:, None]
    qi = np.arange(128, dtype=np.int64)[None, :]
    mask = (kvi <= qi).astype(f32).astype(bf)  # [128, 128] lower-tri incl diag
    rperm = np.zeros((128, 128), f32)
    for r in range(64):
        rperm[2 * r, 2 * r + 1] = -1.0
        rperm[2 * r + 1, 2 * r] = 1.0
    rpermT = np.ascontiguousarray(rperm.T)
    ident = np.eye(128, dtype=f32)
    ones = np.ones((128, 1), bf)
    onescol = np.ones((1, 128), f32)

    in_maps = []
    for c in range(NCORES):
        wqkv = np.concatenate(
            [
                wq[512 * c : 512 * (c + 1), :].T,
                wk[128 * c : 128 * (c + 1), :].T,
                wv[128 * c : 128 * (c + 1), :].T,
            ],
            axis=1,
        ).astype(bf)  # [DIM, 768]
        wot = np.ascontiguousarray(wo[512 * c : 512 * (c + 1), :].T).astype(bf)
        in_maps.append(
            {
                "xt": xt,
                "wqkvt": np.ascontiguousarray(wqkv),
                "wot": wot,
                "cost": cos,
                "sint": sin,
                "maskt": np.ascontiguousarray(mask),
                "rpermt": rpermT,
                "identt": ident,
                "onest": ones,
                "onescolt": onescol,
            }
        )
    return in_maps


def install_ntff_hook():
    """Inject the missing ``antenv.axon_hooks`` module backed by ctypes calls
    into libaxon_pjrt.so, enabling run_bass_kernel_spmd(trace=True) under
    axon. Also neuter upload_artifacts (no artifact bucket here)."""
    import sys as _sys
    import types
    import ctypes
    import contextlib

    if "antenv.axon_hooks" in _sys.modules:
        return
    so_path = "/opt/axon/libaxon_pjrt.so"
    lib = ctypes.CDLL(so_path)
    lib.axon_start_nrt_profile.argtypes = [
        ctypes.POINTER(ctypes.c_int64),
        ctypes.c_size_t,
    ]
    lib.axon_start_nrt_profile.restype = ctypes.c_int64
    lib.axon_stop_nrt_profile.argtypes = [ctypes.c_char_p]
    lib.axon_stop_nrt_profile.restype = ctypes.c_int64

    @contextlib.contextmanager
    def _hook(output_dir, device_ids):
        import jax

        jax.devices()
        if device_ids:
            ids = (ctypes.c_int64 * len(device_ids))(*device_ids)
            rc = lib.axon_start_nrt_profile(ids, len(device_ids))
        else:
            rc = lib.axon_start_nrt_profile(None, 0)
        if rc != 0:
            raise RuntimeError(f"axon_start_nrt_profile rc={rc}")
        try:
            yield
        finally:
            n = lib.axon_stop_nrt_profile(str(output_dir).encode())
            print(f"ntff profile: {n} file(s) written to {output_dir}")

    mod = types.ModuleType("antenv.axon_hooks")
    mod.get_axon_ntff_profile_hook = lambda: _hook
    mod.set_axon_ntff_profile_hook = lambda h: None
    _sys.modules["antenv.axon_hooks"] = mod
    import antenv

    antenv.axon_hooks = mod
    bass_utils.upload_artifacts = lambda tmpdir: tmpdir


def run(x, freqs_cis, wq, wk, wv, wo, trace=False, trace_kwargs=None):
    if trace:
        install_ntff_hook()
    nc = build_nc()
    in_maps = make_in_maps(x, freqs_cis, wq, wk, wv, wo)
    res = bass_utils.run_bass_kernel_spmd(
        nc,
        in_maps,
        core_ids=list(range(NCORES)),
        trace=trace,
        **(trace_kwargs or {}),
    )
    outs = [r["outt"] for r in res.results]  # each [512, S] = outT slice
    full = np.concatenate([np.asarray(o).T for o in outs], axis=1).astype(np.float32)
    return full, res


def kernel(x, freqs_cis, wq, wk, wv, wo):
    full, _ = run(
        np.asarray(x, np.float32),
        np.asarray(freqs_cis, np.float32),
        np.asarray(wq, np.float32),
        np.asarray(wk, np.float32),
        np.asarray(wv, np.float32),
        np.asarray(wo, np.float32),
    )
    return full
